# revision 25
# baseline (speedup 1.0000x reference)
"""Trainium2 Bass kernel for a dense transformer block (pre-LN MHA + MLP).

Data-parallel over batch: batch b -> NeuronCore b (8 cores, no collectives).
Weights are host-pre-transposed into contraction-major layout and cast to
bf16 (layout/precision prep only; all model math runs on device).
"""

import functools
import os

import numpy as np
import ml_dtypes

import concourse.bass as bass
import concourse.bacc as bacc
import concourse.tile as tile
from concourse import mybir
from concourse.bass_utils import run_bass_kernel_spmd

F32 = mybir.dt.float32
BF16 = mybir.dt.bfloat16
NPBF16 = ml_dtypes.bfloat16
AF = mybir.ActivationFunctionType
ALU = mybir.AluOpType

B = 8
N = 1024  # tokens per batch
C = 1024  # model dim
O = 3 * C  # qkv out dim
F = 4096  # mlp hidden
H = 16  # heads
HD = 64  # head dim
P = 128
NT = N // P  # 8 token tiles
CT = C // P  # 8 channel tiles
FT = F // P  # 32 hidden tiles
EPS = 1e-5
SCALE = HD ** -0.5

# stashed by kernel() for test harnesses
last_result = None


def _build(ln1_affine, ln2_affine):
    nc = bacc.Bacc()

    x_d = nc.dram_tensor("x", [N, C], F32, kind="ExternalInput")
    wqkv_d = nc.dram_tensor("wqkvT", [C, O], BF16, kind="ExternalInput")
    wp_d = nc.dram_tensor("wpT", [C, C], BF16, kind="ExternalInput")
    w1_d = nc.dram_tensor("w1T", [C, F], BF16, kind="ExternalInput")
    w2_d = nc.dram_tensor("w2T", [F, C], BF16, kind="ExternalInput")
    pb_d = nc.dram_tensor("proj_b_bf", [1, C], BF16, kind="ExternalInput")
    b1_d = nc.dram_tensor("fc1_b_r", [P, FT], F32, kind="ExternalInput")
    b2_d = nc.dram_tensor("fc2_b_bf", [1, C], BF16, kind="ExternalInput")
    if ln1_affine:
        n1w_d = nc.dram_tensor("norm1_w", [C], F32, kind="ExternalInput")
        n1b_d = nc.dram_tensor("norm1_b", [C], F32, kind="ExternalInput")
    if ln2_affine:
        n2w_d = nc.dram_tensor("norm2_w", [C], F32, kind="ExternalInput")
        n2b_d = nc.dram_tensor("norm2_b", [C], F32, kind="ExternalInput")

    q_d = nc.dram_tensor("q", [N, C], BF16, kind="ExternalOutput")
    k_d = nc.dram_tensor("k", [N, C], BF16, kind="ExternalOutput")
    v_d = nc.dram_tensor("v", [N, C], BF16, kind="ExternalOutput")
    xo_d = nc.dram_tensor("x_out", [N, C], F32, kind="ExternalOutput")
    ho_d = nc.dram_tensor("h_out", [N, C], F32, kind="ExternalOutput")

    def bcast_ap(handle, n):
        a = handle[:]
        return bass.AP(tensor=a.tensor, offset=0, ap=[[0, P], [1, n]])

    with tile.TileContext(nc) as tc:
        with (
            tc.tile_pool(name="base", bufs=1) as base,
            tc.tile_pool(name="ln_tmp", bufs=3) as ln_tmp,
        ):
            # ---------- resident tiles ----------
            x_sb = base.tile([P, NT, C], F32)  # 4MB: x, later becomes x2
            eps_sb = base.tile([P, 1], F32)
            nc.vector.memset(eps_sb, EPS)
            ones1 = base.tile([1, P], BF16)
            nc.vector.memset(ones1, 1.0)
            pbr = base.tile([1, C], BF16)
            nc.sync.dma_start(out=pbr, in_=pb_d[:, :])
            b2r = base.tile([1, C], BF16)
            nc.sync.dma_start(out=b2r, in_=b2_d[:, :])
            b1r = base.tile([P, FT], F32)
            nc.sync.dma_start(out=b1r, in_=b1_d[:, :])
            n1wb = n1bb = n2wb = n2bb = None
            if ln1_affine:
                n1wb = base.tile([P, C], F32)
                nc.gpsimd.dma_start(out=n1wb, in_=bcast_ap(n1w_d, C))
                n1bb = base.tile([P, C], F32)
                nc.gpsimd.dma_start(out=n1bb, in_=bcast_ap(n1b_d, C))
            if ln2_affine:
                n2wb = base.tile([P, C], F32)
                nc.gpsimd.dma_start(out=n2wb, in_=bcast_ap(n2w_d, C))
                n2bb = base.tile([P, C], F32)
                nc.gpsimd.dma_start(out=n2bb, in_=bcast_ap(n2b_d, C))

            def layernorm_to(j, out_bf, affine, wb, bb):
                """LN of x_sb[:, j, :] -> out_bf (bf16 [P, C] natural)."""
                stats = ln_tmp.tile([P, 2, 6], F32)
                for g in range(2):
                    nc.vector.bn_stats(
                        out=stats[:, g, :], in_=x_sb[:, j, g * 512 : (g + 1) * 512]
                    )
                mv = ln_tmp.tile([P, 2], F32)
                nc.vector.bn_aggr(out=mv, in_=stats)
                std = ln_tmp.tile([P, 1], F32)
                nc.scalar.activation(
                    out=std, in_=mv[:, 1:2], func=AF.Sqrt, bias=eps_sb
                )
                rstd = ln_tmp.tile([P, 1], F32)
                nc.vector.reciprocal(out=rstd, in_=std)
                if affine:
                    t0 = ln_tmp.tile([P, C], F32)
                    nc.vector.tensor_scalar(
                        out=t0, in0=x_sb[:, j, :], scalar1=mv[:, 0:1], scalar2=rstd,
                        op0=ALU.subtract, op1=ALU.mult,
                    )
                    t1 = ln_tmp.tile([P, C], F32)
                    nc.vector.tensor_mul(out=t1, in0=t0, in1=wb)
                    nc.vector.tensor_add(out=out_bf, in0=t1, in1=bb)
                else:
                    nc.vector.tensor_scalar(
                        out=out_bf, in0=x_sb[:, j, :], scalar1=mv[:, 0:1],
                        scalar2=rstd, op0=ALU.subtract, op1=ALU.mult,
                    )

            with tc.tile_pool(name="attn_res", bufs=1) as attn_res:
                qt = attn_res.tile([P, CT, N], BF16)  # q transposed [c, n]
                kt = attn_res.tile([P, CT, N], BF16)
                v1 = attn_res.tile([P, NT, H, HD + 1], BF16)  # v natural + ones
                nc.vector.memset(v1[:, :, :, HD : HD + 1], 1.0)
                aot = [
                    attn_res.tile([P, N], BF16, name=f"aot{a}")
                    for a in range(CT)
                ]  # attn out transposed, per head-pair
                wp = attn_res.tile([P, CT, C], BF16)

                # ---------- phase 1: load x, LN1, transpose, QKV ----------
                with (
                    tc.tile_pool(name="qkvw", bufs=3) as qkvw,
                    tc.tile_pool(name="xntp", bufs=1) as xntp,
                    tc.tile_pool(name="qkbf", bufs=6) as qkbf,
                    tc.tile_pool(name="mmA", bufs=3, space="PSUM") as mmA,
                ):
                    xnt = xntp.tile([P, CT, N], BF16)  # xn transposed
                    for j in range(NT):
                        nc.sync.dma_start(
                            out=x_sb[:, j, :], in_=x_d[j * P : (j + 1) * P, :]
                        )
                        xn_bf = ln_tmp.tile([P, C], BF16, tag="xnbf")
                        layernorm_to(j, xn_bf, ln1_affine, n1wb, n1bb)
                        nc.sync.dma_start(
                            out=xnt[:, :, j * P : (j + 1) * P],
                            in_=xn_bf,
                            transpose=True,
                        )

                    for which in range(3):  # 0=q 1=k 2=v
                        wc = qkvw.tile(
                            [P, CT, N], BF16, tag="wc", name=f"wc{which}"
                        )
                        nc.sync.dma_start(
                            out=wc,
                            in_=wqkv_d[:, which * N : (which + 1) * N].rearrange(
                                "(t p) o -> p t o", p=P
                            ),
                        )
                        for j in range(NT):
                            qk = (
                                qkbf.tile([P, C], BF16, name=f"qk_{which}_{j}", tag="qk")
                                if which < 2
                                else None
                            )
                            for half in range(2):
                                ps = mmA.tile([P, 512], F32)
                                for a in range(CT):
                                    nc.tensor.matmul(
                                        ps,
                                        xnt[:, a, j * P : (j + 1) * P],
                                        wc[:, a, half * 512 : (half + 1) * 512],
                                        start=(a == 0),
                                        stop=(a == CT - 1),
                                    )
                                hsl = slice(half * 512, (half + 1) * 512)
                                # bf16 drain (attention operand + output source)
                                if which < 2:
                                    nc.any.tensor_copy(out=qk[:, hsl], in_=ps)
                                else:
                                    nc.any.tensor_copy(
                                        out=v1[
                                            :, j, half * 8 : (half + 1) * 8, 0:HD
                                        ],
                                        in_=ps,
                                    )
                            dst = (q_d, k_d, v_d)[which]
                            if which < 2:
                                # output via SWDGE cast-DMA (bf16 -> f32)
                                nc.gpsimd.dma_start(
                                    out=dst[j * P : (j + 1) * P, :], in_=qk
                                )
                                dstt = (qt, kt)[which]
                                nc.sync.dma_start(
                                    out=dstt[:, :, j * P : (j + 1) * P],
                                    in_=qk,
                                    transpose=True,
                                )
                            else:
                                nc.gpsimd.dma_start(
                                    out=dst[j * P : (j + 1) * P, :],
                                    in_=v1[:, j, :, 0:HD],
                                )

                # ---------- phase 2: attention ----------
                with (
                    tc.tile_pool(name="ptp", bufs=3) as ptp,
                    tc.tile_pool(name="bcp", bufs=3) as bcp,
                    tc.tile_pool(name="spp", bufs=2, space="PSUM") as spp,
                    tc.tile_pool(name="aop", bufs=1, space="PSUM") as aop,
                    tc.tile_pool(name="mmP", bufs=2, space="PSUM") as mmP,
                ):
                    for h in range(H):
                        hb = (h % 2) * 64
                        ha = h // 2
                        ao_ps = aop.tile([HD + 1, N], F32)
                        for mj in range(NT):
                            s_ps = spp.tile([P, N], F32)
                            for nch in range(2):
                                nc.tensor.matmul(
                                    s_ps[:, nch * 512 : (nch + 1) * 512],
                                    kt[hb : hb + 64, ha, mj * P : (mj + 1) * P],
                                    qt[hb : hb + 64, ha, nch * 512 : (nch + 1) * 512],
                                    start=True,
                                    stop=True,
                                )
                            pt = ptp.tile([P, N], BF16)
                            nc.scalar.activation(
                                out=pt, in_=s_ps, func=AF.Exp, scale=SCALE
                            )
                            for nch in range(2):
                                nc.tensor.matmul(
                                    ao_ps[:, nch * 512 : (nch + 1) * 512],
                                    v1[:, mj, h, :],
                                    pt[:, nch * 512 : (nch + 1) * 512],
                                    start=(mj == 0),
                                    stop=(mj == NT - 1),
                                )
                        rden = bcp.tile([1, N], F32, tag="rden")
                        nc.vector.reciprocal(out=rden, in_=ao_ps[HD : HD + 1, :])
                        bc = bcp.tile([64, N], F32, tag="bc")
                        nc.gpsimd.partition_broadcast(bc, rden)
                        nc.vector.tensor_mul(
                            out=aot[ha][hb : hb + 64, :], in0=ao_ps[0:HD, :], in1=bc
                        )

                    # ---------- phase 3: proj + residual ----------
                    for t in range(CT):
                        nc.sync.dma_start(
                            out=wp[:, t, :], in_=wp_d[t * P : (t + 1) * P, :]
                        )
                    for j in range(NT):
                        nc.vector.tensor_add(
                            out=x_sb[:, j, :], in0=x_sb[:, j, :], in1=pbb
                        )
                    for j in range(NT):
                        for oc in range(2):
                            ps = mmP.tile([P, 512], F32)
                            for a in range(CT):
                                nc.tensor.matmul(
                                    ps,
                                    aot[:, a, j * P : (j + 1) * P],
                                    wp[:, a, oc * 512 : (oc + 1) * 512],
                                    start=(a == 0),
                                    stop=(a == CT - 1),
                                )
                            sl = slice(oc * 512, (oc + 1) * 512)
                            nc.vector.tensor_add(
                                out=x_sb[:, j, sl], in0=x_sb[:, j, sl], in1=ps
                            )

            # ---------- phase 4+5: LN2 + MLP ----------
            with (
                tc.tile_pool(name="mlp_res", bufs=1) as mlp_res,
                tc.tile_pool(name="w1p", bufs=2) as w1p,
                tc.tile_pool(name="outp", bufs=3) as outp,
                tc.tile_pool(name="mmB", bufs=4, space="PSUM") as mmB,
            ):
                xn2t = mlp_res.tile([P, CT, N], BF16)
                for j in range(NT):
                    xn2_bf = ln_tmp.tile([P, C], BF16, tag="xnbf")
                    layernorm_to(j, xn2_bf, ln2_affine, n2wb, n2bb)
                    nc.sync.dma_start(
                        out=xn2t[:, :, j * P : (j + 1) * P],
                        in_=xn2_bf,
                        transpose=True,
                    )

                w2 = mlp_res.tile([P, FT, C], BF16)  # 8MB resident
                for t in range(FT):
                    nc.gpsimd.dma_start(
                        out=w2[:, t, :], in_=w2_d[t * P : (t + 1) * P, :]
                    )
                ht = mlp_res.tile([P, FT, 512], BF16)  # one n-chunk at a time

                for nch in range(2):
                    nsl = slice(nch * 512, (nch + 1) * 512)
                    # FC1 + gelu
                    for fc in range(F // 512):
                        w1c = w1p.tile([P, CT, 512], BF16, tag="w1c", name=f"w1c_{nch}_{fc}")
                        nc.sync.dma_start(
                            out=w1c,
                            in_=w1_d[:, fc * 512 : (fc + 1) * 512].rearrange(
                                "(t p) f -> p t f", p=P
                            ),
                        )
                        for fi in range(4):
                            f = fc * 4 + fi
                            ps = mmB.tile([P, 512], F32)
                            for a in range(CT):
                                nc.tensor.matmul(
                                    ps,
                                    w1c[:, a, fi * P : (fi + 1) * P],
                                    xn2t[:, a, nsl],
                                    start=(a == 0),
                                    stop=(a == CT - 1),
                                )
                            nc.scalar.activation(
                                out=ht[:, f, :], in_=ps, func=AF.Gelu,
                                bias=b1r[:, f : f + 1],
                            )
                    # FC2 for the 4 token tiles of this n-chunk
                    for ji in range(4):
                        j = nch * 4 + ji
                        for cc in range(2):
                            ps = mmB.tile([P, 512], F32)
                            for f in range(FT):
                                nc.tensor.matmul(
                                    ps,
                                    ht[:, f, ji * P : (ji + 1) * P],
                                    w2[:, f, cc * 512 : (cc + 1) * 512],
                                    start=(f == 0),
                                    stop=(f == FT - 1),
                                )
                            csl = slice(cc * 512, (cc + 1) * 512)
                            ho_sb = outp.tile([P, 512], F32, tag="ho")
                            nc.vector.tensor_add(out=ho_sb, in0=ps, in1=b2b[:, csl])
                            nc.sync.dma_start(
                                out=ho_d[j * P : (j + 1) * P, csl], in_=ho_sb
                            )
                            xo_sb = outp.tile([P, 512], F32, tag="xo")
                            nc.vector.tensor_add(
                                out=xo_sb, in0=x_sb[:, j, csl], in1=ho_sb
                            )
                            nc.sync.dma_start(
                                out=xo_d[j * P : (j + 1) * P, csl], in_=xo_sb
                            )

    nc.compile()
    return nc


@functools.lru_cache(maxsize=4)
def _build_cached(ln1_affine, ln2_affine):
    return _build(ln1_affine, ln2_affine)


def kernel(x, norm1_w, norm1_b, qkv_w, proj_w, proj_b,
           norm2_w, norm2_b, fc1_w, fc1_b, fc2_w, fc2_b):
    global last_result
    x = np.asarray(x, np.float32)
    ln1_affine = not (
        np.all(np.asarray(norm1_w) == 1.0) and np.all(np.asarray(norm1_b) == 0.0)
    )
    ln2_affine = not (
        np.all(np.asarray(norm2_w) == 1.0) and np.all(np.asarray(norm2_b) == 0.0)
    )

    # host-side layout prep (contraction-major transposes + bf16 cast)
    wqkvT = np.ascontiguousarray(np.asarray(qkv_w, np.float32).T).astype(NPBF16)
    wpT = np.ascontiguousarray(np.asarray(proj_w, np.float32).T).astype(NPBF16)
    w1T = np.ascontiguousarray(np.asarray(fc1_w, np.float32).T).astype(NPBF16)
    w2T = np.ascontiguousarray(np.asarray(fc2_w, np.float32).T).astype(NPBF16)
    b1r = np.ascontiguousarray(np.asarray(fc1_b, np.float32).reshape(FT, P).T)
    pb = np.asarray(proj_b, np.float32).reshape(1, C).astype(NPBF16)
    b2 = np.asarray(fc2_b, np.float32).reshape(1, C).astype(NPBF16)

    nc = _build_cached(ln1_affine, ln2_affine)

    in_maps = []
    for b in range(B):
        m = {
            "x": np.ascontiguousarray(x[b]),
            "wqkvT": wqkvT, "wpT": wpT, "w1T": w1T, "w2T": w2T,
            "proj_b_bf": pb, "fc1_b_r": b1r, "fc2_b_bf": b2,
        }
        if ln1_affine:
            m["norm1_w"] = np.asarray(norm1_w, np.float32)
            m["norm1_b"] = np.asarray(norm1_b, np.float32)
        if ln2_affine:
            m["norm2_w"] = np.asarray(norm2_w, np.float32)
            m["norm2_b"] = np.asarray(norm2_b, np.float32)
        in_maps.append(m)

    trace = os.environ.get("KBLOCK_TRACE", "0") == "1"
    res = run_bass_kernel_spmd(
        nc, in_maps, core_ids=list(range(B)), trace=trace
    )
    last_result = res

    xo = np.stack([res.results[b]["x_out"] for b in range(B)])
    q = np.stack([res.results[b]["q"] for b in range(B)]).astype(np.float32)
    k = np.stack([res.results[b]["k"] for b in range(B)]).astype(np.float32)
    v = np.stack([res.results[b]["v"] for b in range(B)]).astype(np.float32)
    ho = np.stack([res.results[b]["h_out"] for b in range(B)])
    return (xo, (q, k, v), ho)


# revision 26
# speedup vs baseline: 1.1078x; 1.1078x over previous
"""Trainium2 Bass kernel for a dense transformer block (pre-LN MHA + MLP).

Data-parallel over batch: batch b -> NeuronCore b (8 cores, no collectives).
Weights are host-pre-transposed into contraction-major layout and cast to
bf16 (layout/precision prep only; all model math runs on device).
"""

import functools
import os

import numpy as np
import ml_dtypes

import concourse.bass as bass
import concourse.bacc as bacc
import concourse.tile as tile
from concourse import mybir
from concourse.bass_utils import run_bass_kernel_spmd

F32 = mybir.dt.float32
BF16 = mybir.dt.bfloat16
NPBF16 = ml_dtypes.bfloat16
AF = mybir.ActivationFunctionType
ALU = mybir.AluOpType

B = 8
N = 1024  # tokens per batch
C = 1024  # model dim
O = 3 * C  # qkv out dim
F = 4096  # mlp hidden
H = 16  # heads
HD = 64  # head dim
P = 128
NT = N // P  # 8 token tiles
CT = C // P  # 8 channel tiles
FT = F // P  # 32 hidden tiles
EPS = 1e-5
SCALE = HD ** -0.5

# stashed by kernel() for test harnesses
last_result = None


def _build(ln1_affine, ln2_affine):
    nc = bacc.Bacc()

    x_d = nc.dram_tensor("x", [N, C], F32, kind="ExternalInput")
    wqkv_d = nc.dram_tensor("wqkvT", [C, O], BF16, kind="ExternalInput")
    wp_d = nc.dram_tensor("wpT", [C, C], BF16, kind="ExternalInput")
    w1_d = nc.dram_tensor("w1T", [C, F], BF16, kind="ExternalInput")
    w2_d = nc.dram_tensor("w2T", [F, C], BF16, kind="ExternalInput")
    pb_d = nc.dram_tensor("proj_b_bf", [1, C], BF16, kind="ExternalInput")
    b1_d = nc.dram_tensor("fc1_b_r", [P, FT], F32, kind="ExternalInput")
    b2_d = nc.dram_tensor("fc2_b_bf", [1, C], BF16, kind="ExternalInput")
    if ln1_affine:
        n1w_d = nc.dram_tensor("norm1_w", [C], F32, kind="ExternalInput")
        n1b_d = nc.dram_tensor("norm1_b", [C], F32, kind="ExternalInput")
    if ln2_affine:
        n2w_d = nc.dram_tensor("norm2_w", [C], F32, kind="ExternalInput")
        n2b_d = nc.dram_tensor("norm2_b", [C], F32, kind="ExternalInput")

    q_d = nc.dram_tensor("q", [N, C], BF16, kind="ExternalOutput")
    k_d = nc.dram_tensor("k", [N, C], BF16, kind="ExternalOutput")
    v_d = nc.dram_tensor("v", [N, C], BF16, kind="ExternalOutput")
    xo_d = nc.dram_tensor("x_out", [N, C], F32, kind="ExternalOutput")
    ho_d = nc.dram_tensor("h_out", [N, C], F32, kind="ExternalOutput")

    def bcast_ap(handle, n):
        a = handle[:]
        return bass.AP(tensor=a.tensor, offset=0, ap=[[0, P], [1, n]])

    with tile.TileContext(nc) as tc:
        with (
            tc.tile_pool(name="base", bufs=1) as base,
            tc.tile_pool(name="ln_tmp", bufs=3) as ln_tmp,
        ):
            # ---------- resident tiles ----------
            x_sb = base.tile([P, NT, C], F32)  # 4MB: x, later becomes x2
            eps_sb = base.tile([P, 1], F32)
            nc.vector.memset(eps_sb, EPS)
            ones1 = base.tile([1, P], BF16)
            nc.vector.memset(ones1, 1.0)
            pbr = base.tile([1, C], BF16)
            nc.sync.dma_start(out=pbr, in_=pb_d[:, :])
            b2r = base.tile([1, C], BF16)
            nc.sync.dma_start(out=b2r, in_=b2_d[:, :])
            b1r = base.tile([P, FT], F32)
            nc.sync.dma_start(out=b1r, in_=b1_d[:, :])
            n1wb = n1bb = n2wb = n2bb = None
            if ln1_affine:
                n1wb = base.tile([P, C], F32)
                nc.gpsimd.dma_start(out=n1wb, in_=bcast_ap(n1w_d, C))
                n1bb = base.tile([P, C], F32)
                nc.gpsimd.dma_start(out=n1bb, in_=bcast_ap(n1b_d, C))
            if ln2_affine:
                n2wb = base.tile([P, C], F32)
                nc.gpsimd.dma_start(out=n2wb, in_=bcast_ap(n2w_d, C))
                n2bb = base.tile([P, C], F32)
                nc.gpsimd.dma_start(out=n2bb, in_=bcast_ap(n2b_d, C))

            def layernorm_to(j, out_bf, affine, wb, bb):
                """LN of x_sb[:, j, :] -> out_bf (bf16 [P, C] natural)."""
                stats = ln_tmp.tile([P, 2, 6], F32)
                for g in range(2):
                    nc.vector.bn_stats(
                        out=stats[:, g, :], in_=x_sb[:, j, g * 512 : (g + 1) * 512]
                    )
                mv = ln_tmp.tile([P, 2], F32)
                nc.vector.bn_aggr(out=mv, in_=stats)
                std = ln_tmp.tile([P, 1], F32)
                nc.scalar.activation(
                    out=std, in_=mv[:, 1:2], func=AF.Sqrt, bias=eps_sb
                )
                rstd = ln_tmp.tile([P, 1], F32)
                nc.vector.reciprocal(out=rstd, in_=std)
                if affine:
                    t0 = ln_tmp.tile([P, C], F32)
                    nc.vector.tensor_scalar(
                        out=t0, in0=x_sb[:, j, :], scalar1=mv[:, 0:1], scalar2=rstd,
                        op0=ALU.subtract, op1=ALU.mult,
                    )
                    t1 = ln_tmp.tile([P, C], F32)
                    nc.vector.tensor_mul(out=t1, in0=t0, in1=wb)
                    nc.vector.tensor_add(out=out_bf, in0=t1, in1=bb)
                else:
                    nc.vector.tensor_scalar(
                        out=out_bf, in0=x_sb[:, j, :], scalar1=mv[:, 0:1],
                        scalar2=rstd, op0=ALU.subtract, op1=ALU.mult,
                    )

            with tc.tile_pool(name="attn_res", bufs=1) as attn_res:
                qt = attn_res.tile([P, CT, N], BF16)  # q transposed [c, n]
                kt = attn_res.tile([P, CT, N], BF16)
                v1 = attn_res.tile([P, NT, H, HD + 1], BF16)  # v natural + ones
                nc.vector.memset(v1[:, :, :, HD : HD + 1], 1.0)
                wp = attn_res.tile([P, CT, C], BF16)

                # ---------- phase 1: load x, LN1, transpose, QKV ----------
                with (
                    tc.tile_pool(name="qkvw", bufs=3) as qkvw,
                    tc.tile_pool(name="xntp", bufs=1) as xntp,
                    tc.tile_pool(name="qkbf", bufs=6) as qkbf,
                    tc.tile_pool(name="mmA", bufs=3, space="PSUM") as mmA,
                ):
                    xnt = xntp.tile([P, CT, N], BF16)  # xn transposed
                    for j in range(NT):
                        nc.sync.dma_start(
                            out=x_sb[:, j, :], in_=x_d[j * P : (j + 1) * P, :]
                        )
                        xn_bf = ln_tmp.tile([P, C], BF16, tag="xnbf")
                        layernorm_to(j, xn_bf, ln1_affine, n1wb, n1bb)
                        nc.sync.dma_start(
                            out=xnt[:, :, j * P : (j + 1) * P],
                            in_=xn_bf,
                            transpose=True,
                        )

                    for which in range(3):  # 0=q 1=k 2=v
                        wc = qkvw.tile(
                            [P, CT, N], BF16, tag="wc", name=f"wc{which}"
                        )
                        nc.sync.dma_start(
                            out=wc,
                            in_=wqkv_d[:, which * N : (which + 1) * N].rearrange(
                                "(t p) o -> p t o", p=P
                            ),
                        )
                        for j in range(NT):
                            qk = (
                                qkbf.tile([P, C], BF16, name=f"qk_{which}_{j}", tag="qk")
                                if which < 2
                                else None
                            )
                            for half in range(2):
                                ps = mmA.tile([P, 512], F32)
                                for a in range(CT):
                                    nc.tensor.matmul(
                                        ps,
                                        xnt[:, a, j * P : (j + 1) * P],
                                        wc[:, a, half * 512 : (half + 1) * 512],
                                        start=(a == 0),
                                        stop=(a == CT - 1),
                                    )
                                hsl = slice(half * 512, (half + 1) * 512)
                                # bf16 drain (attention operand + output source)
                                if which < 2:
                                    nc.any.tensor_copy(out=qk[:, hsl], in_=ps)
                                else:
                                    nc.any.tensor_copy(
                                        out=v1[
                                            :, j, half * 8 : (half + 1) * 8, 0:HD
                                        ],
                                        in_=ps,
                                    )
                            dst = (q_d, k_d, v_d)[which]
                            if which < 2:
                                # output via SWDGE cast-DMA (bf16 -> f32)
                                nc.gpsimd.dma_start(
                                    out=dst[j * P : (j + 1) * P, :], in_=qk
                                )
                                dstt = (qt, kt)[which]
                                nc.sync.dma_start(
                                    out=dstt[:, :, j * P : (j + 1) * P],
                                    in_=qk,
                                    transpose=True,
                                )
                            else:
                                nc.gpsimd.dma_start(
                                    out=dst[j * P : (j + 1) * P, :],
                                    in_=v1[:, j, :, 0:HD],
                                )

                # ---------- phase 2: attention ----------
                with (
                    tc.tile_pool(name="ptp", bufs=3) as ptp,
                    tc.tile_pool(name="bcp", bufs=3) as bcp,
                    tc.tile_pool(name="spp", bufs=2, space="PSUM") as spp,
                    tc.tile_pool(name="aop", bufs=1, space="PSUM") as aop,
                    tc.tile_pool(name="mmP", bufs=2, space="PSUM") as mmP,
                ):
                    for h in range(H):
                        hb = (h % 2) * 64
                        ha = h // 2
                        ao_ps = aop.tile([HD + 1, N], F32)
                        for mj in range(NT):
                            s_ps = spp.tile([P, N], F32)
                            for nch in range(2):
                                nc.tensor.matmul(
                                    s_ps[:, nch * 512 : (nch + 1) * 512],
                                    kt[hb : hb + 64, ha, mj * P : (mj + 1) * P],
                                    qt[hb : hb + 64, ha, nch * 512 : (nch + 1) * 512],
                                    start=True,
                                    stop=True,
                                )
                            pt = ptp.tile([P, N], BF16)
                            nc.scalar.activation(
                                out=pt, in_=s_ps, func=AF.Exp, scale=SCALE
                            )
                            for nch in range(2):
                                nc.tensor.matmul(
                                    ao_ps[:, nch * 512 : (nch + 1) * 512],
                                    v1[:, mj, h, :],
                                    pt[:, nch * 512 : (nch + 1) * 512],
                                    start=(mj == 0),
                                    stop=(mj == NT - 1),
                                )
                        rden = bcp.tile([1, N], F32, tag="rden")
                        nc.vector.reciprocal(out=rden, in_=ao_ps[HD : HD + 1, :])
                        bc = bcp.tile([64, N], F32, tag="bc")
                        nc.gpsimd.partition_broadcast(bc, rden)
                        nc.vector.tensor_mul(
                            out=aot[ha][hb : hb + 64, :], in0=ao_ps[0:HD, :], in1=bc
                        )

                    # ---------- phase 3: proj + residual ----------
                    for t in range(CT):
                        nc.sync.dma_start(
                            out=wp[:, t, :], in_=wp_d[t * P : (t + 1) * P, :]
                        )
                    for j in range(NT):
                        nc.vector.tensor_add(
                            out=x_sb[:, j, :], in0=x_sb[:, j, :], in1=pbb
                        )
                    for j in range(NT):
                        for oc in range(2):
                            ps = mmP.tile([P, 512], F32)
                            for a in range(CT):
                                nc.tensor.matmul(
                                    ps,
                                    aot[:, a, j * P : (j + 1) * P],
                                    wp[:, a, oc * 512 : (oc + 1) * 512],
                                    start=(a == 0),
                                    stop=(a == CT - 1),
                                )
                            sl = slice(oc * 512, (oc + 1) * 512)
                            nc.vector.tensor_add(
                                out=x_sb[:, j, sl], in0=x_sb[:, j, sl], in1=ps
                            )

            # ---------- phase 4+5: LN2 + MLP ----------
            with (
                tc.tile_pool(name="mlp_res", bufs=1) as mlp_res,
                tc.tile_pool(name="w1p", bufs=2) as w1p,
                tc.tile_pool(name="outp", bufs=3) as outp,
                tc.tile_pool(name="mmB", bufs=4, space="PSUM") as mmB,
            ):
                xn2t = mlp_res.tile([P, CT, N], BF16)
                for j in range(NT):
                    xn2_bf = ln_tmp.tile([P, C], BF16, tag="xnbf")
                    layernorm_to(j, xn2_bf, ln2_affine, n2wb, n2bb)
                    nc.sync.dma_start(
                        out=xn2t[:, :, j * P : (j + 1) * P],
                        in_=xn2_bf,
                        transpose=True,
                    )

                w2 = mlp_res.tile([P, FT, C], BF16)  # 8MB resident
                for t in range(FT):
                    nc.gpsimd.dma_start(
                        out=w2[:, t, :], in_=w2_d[t * P : (t + 1) * P, :]
                    )
                ht = mlp_res.tile([P, FT, 512], BF16)  # one n-chunk at a time

                for nch in range(2):
                    nsl = slice(nch * 512, (nch + 1) * 512)
                    # FC1 + gelu
                    for fc in range(F // 512):
                        w1c = w1p.tile([P, CT, 512], BF16, tag="w1c", name=f"w1c_{nch}_{fc}")
                        nc.sync.dma_start(
                            out=w1c,
                            in_=w1_d[:, fc * 512 : (fc + 1) * 512].rearrange(
                                "(t p) f -> p t f", p=P
                            ),
                        )
                        for fi in range(4):
                            f = fc * 4 + fi
                            ps = mmB.tile([P, 512], F32)
                            for a in range(CT):
                                nc.tensor.matmul(
                                    ps,
                                    w1c[:, a, fi * P : (fi + 1) * P],
                                    xn2t[:, a, nsl],
                                    start=(a == 0),
                                    stop=(a == CT - 1),
                                )
                            nc.scalar.activation(
                                out=ht[:, f, :], in_=ps, func=AF.Gelu,
                                bias=b1r[:, f : f + 1],
                            )
                    # FC2 for the 4 token tiles of this n-chunk
                    for ji in range(4):
                        j = nch * 4 + ji
                        for cc in range(2):
                            ps = mmB.tile([P, 512], F32)
                            for f in range(FT):
                                nc.tensor.matmul(
                                    ps,
                                    ht[:, f, ji * P : (ji + 1) * P],
                                    w2[:, f, cc * 512 : (cc + 1) * 512],
                                    start=(f == 0),
                                    stop=(f == FT - 1),
                                )
                            csl = slice(cc * 512, (cc + 1) * 512)
                            ho_sb = outp.tile([P, 512], F32, tag="ho")
                            nc.vector.tensor_add(out=ho_sb, in0=ps, in1=b2b[:, csl])
                            nc.sync.dma_start(
                                out=ho_d[j * P : (j + 1) * P, csl], in_=ho_sb
                            )
                            xo_sb = outp.tile([P, 512], F32, tag="xo")
                            nc.vector.tensor_add(
                                out=xo_sb, in0=x_sb[:, j, csl], in1=ho_sb
                            )
                            nc.sync.dma_start(
                                out=xo_d[j * P : (j + 1) * P, csl], in_=xo_sb
                            )

    nc.compile()
    return nc


@functools.lru_cache(maxsize=4)
def _build_cached(ln1_affine, ln2_affine):
    return _build(ln1_affine, ln2_affine)


def kernel(x, norm1_w, norm1_b, qkv_w, proj_w, proj_b,
           norm2_w, norm2_b, fc1_w, fc1_b, fc2_w, fc2_b):
    global last_result
    x = np.asarray(x, np.float32)
    ln1_affine = not (
        np.all(np.asarray(norm1_w) == 1.0) and np.all(np.asarray(norm1_b) == 0.0)
    )
    ln2_affine = not (
        np.all(np.asarray(norm2_w) == 1.0) and np.all(np.asarray(norm2_b) == 0.0)
    )

    # host-side layout prep (contraction-major transposes + bf16 cast)
    wqkvT = np.ascontiguousarray(np.asarray(qkv_w, np.float32).T).astype(NPBF16)
    wpT = np.ascontiguousarray(np.asarray(proj_w, np.float32).T).astype(NPBF16)
    w1T = np.ascontiguousarray(np.asarray(fc1_w, np.float32).T).astype(NPBF16)
    w2T = np.ascontiguousarray(np.asarray(fc2_w, np.float32).T).astype(NPBF16)
    b1r = np.ascontiguousarray(np.asarray(fc1_b, np.float32).reshape(FT, P).T)
    pb = np.asarray(proj_b, np.float32).reshape(1, C).astype(NPBF16)
    b2 = np.asarray(fc2_b, np.float32).reshape(1, C).astype(NPBF16)

    nc = _build_cached(ln1_affine, ln2_affine)

    in_maps = []
    for b in range(B):
        m = {
            "x": np.ascontiguousarray(x[b]),
            "wqkvT": wqkvT, "wpT": wpT, "w1T": w1T, "w2T": w2T,
            "proj_b_bf": pb, "fc1_b_r": b1r, "fc2_b_bf": b2,
        }
        if ln1_affine:
            m["norm1_w"] = np.asarray(norm1_w, np.float32)
            m["norm1_b"] = np.asarray(norm1_b, np.float32)
        if ln2_affine:
            m["norm2_w"] = np.asarray(norm2_w, np.float32)
            m["norm2_b"] = np.asarray(norm2_b, np.float32)
        in_maps.append(m)

    trace = os.environ.get("KBLOCK_TRACE", "0") == "1"
    res = run_bass_kernel_spmd(
        nc, in_maps, core_ids=list(range(B)), trace=trace
    )
    last_result = res

    xo = np.stack([res.results[b]["x_out"] for b in range(B)])
    q = np.stack([res.results[b]["q"] for b in range(B)]).astype(np.float32)
    k = np.stack([res.results[b]["k"] for b in range(B)]).astype(np.float32)
    v = np.stack([res.results[b]["v"] for b in range(B)]).astype(np.float32)
    ho = np.stack([res.results[b]["h_out"] for b in range(B)])
    return (xo, (q, k, v), ho)
